# revision 33
# baseline (speedup 1.0000x reference)
"""Trainium2 Bass kernel for nn_EuclideanDistance (retrieval_knn).

out = quantize(x_pad) @ quantize(temp)
  where temp  = [weight; broadcast(bias, L rows)],  bias = colsum(weight^2)/L
        x_pad = [x, ones(B, L)]
        quantize(t) = round(t/s)*s,  s = max(max|t|/127, 1e-12)  (per tensor)

Strategy: shard the stored-vector axis N=16384 across 8 cores (2048 each),
replicate x. Per-tensor scales sx, sw are global scalars computed on host.

Numerics: round(t/s) are integers |k| <= 127, exact in bf16; the integer
matmul accumulates exactly in f32 PSUM (|sum| <= 544*127^2 < 2^24), so the
bf16 PE matmul reproduces the reference fp32 computation to ~1e-5.

The kernel computes out^T (N on partitions): lhsT = quantized weight chunks,
rhs = quantized x^T. In this orientation the contribution of the L ones
columns x the bias rows --- sum_l k1*kb[n] = L*k1*kb[n], constant across B ---
is a per-partition scalar, folded for free into the PSUM-evacuate op
(out = (psum + c) * sx*sw). That removes the ragged 5th K-chunk: K = 4x128.
"""

import sys

import numpy as np

try:
    import concourse.bacc as bacc  # noqa: F401
except ImportError:  # fresh interpreter without the repo on sys.path
    sys.path.insert(0, "/opt/trn_rl_repo")

import concourse.bacc as bacc
import concourse.mybir as mybir
import concourse.tile as tile
from concourse import bass_utils

B, D, N = 1024, 512, 16384
NCORES = 8
NS = N // NCORES          # 2048 stored vectors per core
L = 32                    # split_square_len
QMAX = np.float32(127.0)  # 2**(8-1) - 1
MAGIC = 12582912.0        # 1.5 * 2**23: float32 round-to-nearest-even trick
KC = D // 128             # 4 K-chunks
NC = NS // 128            # 16 output-partition chunks
BT = B // 512             # 2 rhs tiles

F32 = mybir.dt.float32
BF16 = mybir.dt.bfloat16
I8 = mybir.dt.int8

_NC_CACHE = None


def _body(nc, tc, xT, w8, sc, cb, outT):
    from contextlib import ExitStack

    ID = mybir.ActivationFunctionType.Identity
    ADD = mybir.AluOpType.add
    MULT = mybir.AluOpType.mult

    with ExitStack() as ctx:
        cpool = ctx.enter_context(tc.tile_pool(name="const", bufs=1))
        qpool = ctx.enter_context(tc.tile_pool(name="qk", bufs=1))
        spool = ctx.enter_context(tc.tile_pool(name="stage", bufs=3))
        ppool = ctx.enter_context(tc.tile_pool(name="psum", bufs=8, space="PSUM"))
        opool = ctx.enter_context(tc.tile_pool(name="osb", bufs=4))

        scv = cpool.tile([128, 4], F32, name="scv")
        nc.sync.dma_start(scv, sc)
        inv_sx = scv[:, 0:1]
        inv_sw = scv[:, 1:2]
        sxsw = scv[:, 2:3]
        magic = scv[:, 3:4]
        cbv = cpool.tile([128, 2 * NC], F32, name="cbv")

        JB = 512                   # w column-block width
        NJB = NS // JB             # 4 blocks
        JPB = JB // 128            # 4 output chunks per block

        # ---- all input DMAs upfront on the sync ring (strict FIFO): block 0
        #      + x interleaved first, then the remaining w blocks, so no
        #      store ever convoys ahead of a load ----
        # ---- loads, all on the sync HWDGE ring (the scalar ring measures
        #      ~2.5x slower): first-x-half + w8 block 0 first for the
        #      shortest path to the first matmul, then the rest, then
        #      (later, in program order) the output stores ----
        wfs = {}
        xfs = []
        for k in range(KC):
            xf = spool.tile([128, B], F32, name="xf", tag="xf", bufs=4)
            xfs.append(xf)
        nc.sync.dma_start(xfs[0][:, 0:512], xT[0:128, 0:512])
        for k in range(KC):
            wf = spool.tile([128, JB], I8, name="wf", tag=f"wf{k}",
                            bufs=NJB)
            nc.sync.dma_start(wf, w8[k * 128:(k + 1) * 128, 0:JB])
            wfs[(0, k)] = wf
        nc.sync.dma_start(xfs[0][:, 512:B], xT[0:128, 512:B])
        for k in range(1, KC):
            nc.sync.dma_start(xfs[k], xT[k * 128:(k + 1) * 128, :])
        nc.sync.dma_start(cbv, cb)   # needed only by the evacs
        for jb in range(1, NJB):
            for k in range(KC):
                wf = spool.tile([128, JB], I8, name="wf", tag=f"wf{k}",
                                bufs=NJB)
                nc.sync.dma_start(
                    wf, w8[k * 128:(k + 1) * 128, jb * JB:(jb + 1) * JB])
                wfs[(jb, k)] = wf

        # ---- quantize x (device) / convert w int8 -> bf16 ----
        kxs = []
        kwblocks = [[None] * KC for _ in range(NJB)]

        def conv_wblock(jb):
            for k in range(KC):
                kw = spool.tile([128, JB], BF16, name=f"kw{k}",
                                tag=f"kw{k}", bufs=NJB)
                nc.vector.tensor_copy(kw, wfs[(jb, k)])
                kwblocks[jb][k] = kw

        for k in range(KC):
            kw = spool.tile([128, JB], BF16, name=f"kw{k}", tag=f"kw{k}",
                            bufs=NJB)
            nc.vector.tensor_copy(kw, wfs[(0, k)])
            kwblocks[0][k] = kw

            xm = spool.tile([128, B], F32, name="xm", tag="xm", bufs=4)
            kx = qpool.tile([128, B], BF16, name=f"kx{k}", tag=f"kx{k}")
            if k == 0:  # halves, matching the split first load
                for h in range(2):
                    hs = slice(h * 512, (h + 1) * 512)
                    nc.scalar.activation(xm[:, hs], xfs[k][:, hs], ID,
                                         bias=magic, scale=inv_sx)
                    nc.vector.tensor_scalar_add(kx[:, hs], xm[:, hs], -MAGIC)
            else:
                nc.scalar.activation(xm, xfs[k], ID, bias=magic,
                                     scale=inv_sx)
                nc.vector.tensor_scalar_add(kx, xm, -MAGIC)
            kxs.append(kx)

        conv_wblock(1)  # prefetch distance 2: convert ops for block jb+2
        conv_wblock(2)  # are emitted during block jb's compute

        for jb in range(NJB):
            if 2 < jb + 2 < NJB:
                conv_wblock(jb + 2)

            kwb = kwblocks[jb]
            for jp in range(JPB // 2):
                ob = opool.tile([128, 2 * B], F32, name="ob", tag="ob",
                                bufs=3)
                for h in range(2):
                    jj = jp * 2 + h
                    j = jb * JPB + jj
                    ps = ppool.tile([128, B], F32, name="ps", tag="ps",
                                    bufs=4)
                    for k in range(KC):
                        lhsT = kwb[k][:, jj * 128:(jj + 1) * 128]
                        for b in range(BT):
                            nc.tensor.matmul(
                                ps[:, b * 512:(b + 1) * 512], lhsT,
                                kxs[k][:, b * 512:(b + 1) * 512],
                                start=(k == 0), stop=(k == KC - 1))
                    obs = ob[:, h * B:(h + 1) * B]
                    if h == 0:
                        # (psum + c_int) * (sx*sw) on DVE
                        nc.vector.tensor_scalar(obs, ps, cbv[:, j:j + 1],
                                                sxsw, ADD, MULT)
                    else:
                        # psum * (sx*sw) + c_scaled on ACT
                        nc.scalar.activation(obs, ps, ID,
                                             bias=cbv[:, NC + j:NC + j + 1],
                                             scale=sxsw)
                j0 = jb * JPB + jp * 2
                # one 1 MB store for both 128-row chunks: fewer DMA
                # completions on the ring
                nc.sync.dma_start(
                    outT[j0 * 128:(j0 + 2) * 128, :]
                    .rearrange("(a p) c -> p a c", p=128),
                    ob.rearrange("p (a c) -> p a c", a=2))


def _build():
    global _NC_CACHE
    if _NC_CACHE is not None:
        return _NC_CACHE
    nc = bacc.Bacc("TRN2", target_bir_lowering=False, debug=False,
                   enable_asserts=False, num_devices=1)
    xT = nc.dram_tensor("xT", [D, B], F32, kind="ExternalInput").ap()
    w8 = nc.dram_tensor("w8", [D, NS], I8, kind="ExternalInput").ap()
    sc = nc.dram_tensor("sc", [128, 4], F32, kind="ExternalInput").ap()
    cb = nc.dram_tensor("cb", [128, 2 * NC], F32, kind="ExternalInput").ap()
    outT = nc.dram_tensor("outT", [NS, B], F32, kind="ExternalOutput").ap()
    with tile.TileContext(nc) as tc:
        _body(nc, tc, xT, w8, sc, cb, outT)
    nc.compile()
    _NC_CACHE = nc
    return nc


def _prepare_inputs(x, weight, split_square_len):
    assert x.shape == (B, D) and weight.shape == (D, N)
    assert int(split_square_len) == L

    x = np.ascontiguousarray(x, dtype=np.float32)
    weight = np.ascontiguousarray(weight, dtype=np.float32)

    # bias = colsum(weight^2)/L in f32, matching the reference
    bias = (np.einsum("dn,dn->n", weight, weight, dtype=np.float32)
            / np.float32(L)).astype(np.float32)

    # global per-tensor scales (f32 arithmetic to match jax)
    max_x = np.float32(max(np.abs(x).max(), np.float32(1.0)))
    sx = np.maximum(max_x / QMAX, np.float32(1e-12))
    max_w = np.float32(max(np.abs(weight).max(), np.abs(bias).max()))
    sw = np.maximum(max_w / QMAX, np.float32(1e-12))

    x_T = np.ascontiguousarray(x.T)  # [D, B]

    sc = np.zeros((128, 4), dtype=np.float32)
    sc[:, 0] = np.float32(1.0) / sx
    sc[:, 1] = np.float32(1.0) / sw
    sc[:, 2] = sx * sw
    sc[:, 3] = np.float32(MAGIC)

    # ones/bias rank-1 term: c[n] = L * round(1/sx) * round(bias[n]/sw),
    # exact integers; divides (not reciprocal-mults) to match the reference.
    k1 = np.float32(np.round(np.float32(1.0) / sx))
    kb = np.round(bias / sw).astype(np.float32)
    c_int = (np.float32(L) * k1) * kb          # exact in f32 (< 2^24)
    c_scaled = c_int * (sx * sw)

    # stored-vector database, quantized offline (true divide = reference)
    w_q = np.round(weight / sw).astype(np.int8)

    in_maps = []
    for c in range(NCORES):
        sl = slice(c * NS, (c + 1) * NS)
        cb = np.concatenate([
            c_int[sl].reshape(NC, 128).T,      # [128, NC], col j = chunk j
            c_scaled[sl].reshape(NC, 128).T,
        ], axis=1).astype(np.float32)
        cb = np.ascontiguousarray(cb)
        in_maps.append({
            "xT": x_T,
            "w8": np.ascontiguousarray(w_q[:, sl]),
            "sc": sc,
            "cb": cb,
        })
    return in_maps


def _run(in_maps, **kwargs):
    nc = _build()
    return bass_utils.run_bass_kernel_spmd(
        nc, in_maps, core_ids=list(range(NCORES)), **kwargs)


def kernel(x, weight, split_square_len):
    in_maps = _prepare_inputs(x, weight, split_square_len)
    res = _run(in_maps)
    outT = np.concatenate([res.results[c]["outT"] for c in range(NCORES)],
                          axis=0)          # [N, B]
    return outT.T                          # [B, N] view


# revision 34
# speedup vs baseline: 1.0328x; 1.0328x over previous
"""Trainium2 Bass kernel for nn_EuclideanDistance (retrieval_knn).

out = quantize(x_pad) @ quantize(temp)
  where temp  = [weight; broadcast(bias, L rows)],  bias = colsum(weight^2)/L
        x_pad = [x, ones(B, L)]
        quantize(t) = round(t/s)*s,  s = max(max|t|/127, 1e-12)  (per tensor)

Strategy: shard the stored-vector axis N=16384 across 8 cores (2048 each),
replicate x. Per-tensor scales sx, sw are global scalars computed on host.

Numerics: round(t/s) are integers |k| <= 127, exact in bf16; the integer
matmul accumulates exactly in f32 PSUM (|sum| <= 544*127^2 < 2^24), so the
bf16 PE matmul reproduces the reference fp32 computation to ~1e-5.

The kernel computes out^T (N on partitions): lhsT = quantized weight chunks,
rhs = quantized x^T. In this orientation the contribution of the L ones
columns x the bias rows --- sum_l k1*kb[n] = L*k1*kb[n], constant across B ---
is a per-partition scalar, folded for free into the PSUM-evacuate op
(out = (psum + c) * sx*sw). That removes the ragged 5th K-chunk: K = 4x128.
"""

import sys

import numpy as np

try:
    import concourse.bacc as bacc  # noqa: F401
except ImportError:  # fresh interpreter without the repo on sys.path
    sys.path.insert(0, "/opt/trn_rl_repo")

import concourse.bacc as bacc
import concourse.mybir as mybir
import concourse.tile as tile
from concourse import bass_utils

B, D, N = 1024, 512, 16384
NCORES = 8
NS = N // NCORES          # 2048 stored vectors per core
L = 32                    # split_square_len
QMAX = np.float32(127.0)  # 2**(8-1) - 1
MAGIC = 12582912.0        # 1.5 * 2**23: float32 round-to-nearest-even trick
KC = D // 128             # 4 K-chunks
NC = NS // 128            # 16 output-partition chunks
BT = B // 512             # 2 rhs tiles

F32 = mybir.dt.float32
BF16 = mybir.dt.bfloat16
I8 = mybir.dt.int8

_NC_CACHE = None


def _body(nc, tc, xT, w8, sc, cb, outT):
    from contextlib import ExitStack

    ID = mybir.ActivationFunctionType.Identity
    ADD = mybir.AluOpType.add
    MULT = mybir.AluOpType.mult

    with ExitStack() as ctx:
        cpool = ctx.enter_context(tc.tile_pool(name="const", bufs=1))
        qpool = ctx.enter_context(tc.tile_pool(name="qk", bufs=1))
        spool = ctx.enter_context(tc.tile_pool(name="stage", bufs=3))
        ppool = ctx.enter_context(tc.tile_pool(name="psum", bufs=8, space="PSUM"))
        opool = ctx.enter_context(tc.tile_pool(name="osb", bufs=4))

        scv = cpool.tile([128, 4], F32, name="scv")
        nc.sync.dma_start(scv, sc)
        inv_sx = scv[:, 0:1]
        inv_sw = scv[:, 1:2]
        sxsw = scv[:, 2:3]
        magic = scv[:, 3:4]
        cbv = cpool.tile([128, 2 * NC], F32, name="cbv")

        # ---- loads, all on the sync HWDGE ring (the scalar ring measures
        #      ~2.5x slower). Strict FIFO, so: first-x-half and the first
        #      w8 chunk lead (shortest path to the first matmul), stores
        #      trail every load. w8 is int8, 1 MB total. ----
        xfs = []
        wfs = []
        for k in range(KC):
            xf = spool.tile([128, B], F32, name="xf", tag="xf", bufs=4)
            xfs.append(xf)
            wf = spool.tile([128, NS], I8, name="wf", tag=f"wf{k}", bufs=1)
            wfs.append(wf)
        nc.sync.dma_start(xfs[0][:, 0:512], xT[0:128, 0:512])
        nc.sync.dma_start(wfs[0], w8[0:128, :])
        nc.sync.dma_start(xfs[0][:, 512:B], xT[0:128, 512:B])
        nc.sync.dma_start(wfs[1], w8[128:256, :])
        for k in range(1, KC):
            nc.sync.dma_start(xfs[k], xT[k * 128:(k + 1) * 128, :])
            if k + 1 < KC:
                nc.sync.dma_start(wfs[k + 1], w8[(k + 1) * 128:(k + 2) * 128, :])
        nc.sync.dma_start(cbv, cb)   # needed only by the evacs

        # ---- quantize x (device) / convert w int8 -> bf16 ----
        kxs = []
        kws = []
        for k in range(KC):
            kw = qpool.tile([128, NS], BF16, name=f"kw{k}", tag=f"kw{k}")
            nc.vector.tensor_copy(kw, wfs[k])
            kws.append(kw)

            xm = spool.tile([128, B], F32, name="xm", tag="xm", bufs=4)
            kx = qpool.tile([128, B], BF16, name=f"kx{k}", tag=f"kx{k}")
            if k == 0:  # halves, matching the split first load
                for h in range(2):
                    hs = slice(h * 512, (h + 1) * 512)
                    nc.scalar.activation(xm[:, hs], xfs[k][:, hs], ID,
                                         bias=magic, scale=inv_sx)
                    nc.vector.tensor_scalar_add(kx[:, hs], xm[:, hs], -MAGIC)
            else:
                nc.scalar.activation(xm, xfs[k], ID, bias=magic,
                                     scale=inv_sx)
                nc.vector.tensor_scalar_add(kx, xm, -MAGIC)
            kxs.append(kx)

        # ---- 16 output chunks, paired into 1 MB stores ----
        for jp in range(NC // 2):
            ob = opool.tile([128, 2 * B], F32, name="ob", tag="ob", bufs=3)
            for h in range(2):
                j = jp * 2 + h
                ps = ppool.tile([128, B], F32, name="ps", tag="ps", bufs=4)
                for k in range(KC):
                    lhsT = kws[k][:, j * 128:(j + 1) * 128]
                    for b in range(BT):
                        nc.tensor.matmul(
                            ps[:, b * 512:(b + 1) * 512], lhsT,
                            kxs[k][:, b * 512:(b + 1) * 512],
                            start=(k == 0), stop=(k == KC - 1))
                obs = ob[:, h * B:(h + 1) * B]
                if h == 0:
                    # (psum + c_int) * (sx*sw) on DVE
                    nc.vector.tensor_scalar(obs, ps, cbv[:, j:j + 1],
                                            sxsw, ADD, MULT)
                else:
                    # psum * (sx*sw) + c_scaled on ACT
                    nc.scalar.activation(obs, ps, ID,
                                         bias=cbv[:, NC + j:NC + j + 1],
                                         scale=sxsw)
            j0 = jp * 2
            # one 1 MB store for both 128-row chunks: fewer DMA
            # completions on the ring
            nc.sync.dma_start(
                outT[j0 * 128:(j0 + 2) * 128, :]
                .rearrange("(a p) c -> p a c", p=128),
                ob.rearrange("p (a c) -> p a c", a=2))


def _build():
    global _NC_CACHE
    if _NC_CACHE is not None:
        return _NC_CACHE
    nc = bacc.Bacc("TRN2", target_bir_lowering=False, debug=False,
                   enable_asserts=False, num_devices=1)
    xT = nc.dram_tensor("xT", [D, B], F32, kind="ExternalInput").ap()
    w8 = nc.dram_tensor("w8", [D, NS], I8, kind="ExternalInput").ap()
    sc = nc.dram_tensor("sc", [128, 4], F32, kind="ExternalInput").ap()
    cb = nc.dram_tensor("cb", [128, 2 * NC], F32, kind="ExternalInput").ap()
    outT = nc.dram_tensor("outT", [NS, B], F32, kind="ExternalOutput").ap()
    with tile.TileContext(nc) as tc:
        _body(nc, tc, xT, w8, sc, cb, outT)
    nc.compile()
    _NC_CACHE = nc
    return nc


def _prepare_inputs(x, weight, split_square_len):
    assert x.shape == (B, D) and weight.shape == (D, N)
    assert int(split_square_len) == L

    x = np.ascontiguousarray(x, dtype=np.float32)
    weight = np.ascontiguousarray(weight, dtype=np.float32)

    # bias = colsum(weight^2)/L in f32, matching the reference
    bias = (np.einsum("dn,dn->n", weight, weight, dtype=np.float32)
            / np.float32(L)).astype(np.float32)

    # global per-tensor scales (f32 arithmetic to match jax)
    max_x = np.float32(max(np.abs(x).max(), np.float32(1.0)))
    sx = np.maximum(max_x / QMAX, np.float32(1e-12))
    max_w = np.float32(max(np.abs(weight).max(), np.abs(bias).max()))
    sw = np.maximum(max_w / QMAX, np.float32(1e-12))

    x_T = np.ascontiguousarray(x.T)  # [D, B]

    sc = np.zeros((128, 4), dtype=np.float32)
    sc[:, 0] = np.float32(1.0) / sx
    sc[:, 1] = np.float32(1.0) / sw
    sc[:, 2] = sx * sw
    sc[:, 3] = np.float32(MAGIC)

    # ones/bias rank-1 term: c[n] = L * round(1/sx) * round(bias[n]/sw),
    # exact integers; divides (not reciprocal-mults) to match the reference.
    k1 = np.float32(np.round(np.float32(1.0) / sx))
    kb = np.round(bias / sw).astype(np.float32)
    c_int = (np.float32(L) * k1) * kb          # exact in f32 (< 2^24)
    c_scaled = c_int * (sx * sw)

    # stored-vector database, quantized offline (true divide = reference)
    w_q = np.round(weight / sw).astype(np.int8)

    in_maps = []
    for c in range(NCORES):
        sl = slice(c * NS, (c + 1) * NS)
        cb = np.concatenate([
            c_int[sl].reshape(NC, 128).T,      # [128, NC], col j = chunk j
            c_scaled[sl].reshape(NC, 128).T,
        ], axis=1).astype(np.float32)
        cb = np.ascontiguousarray(cb)
        in_maps.append({
            "xT": x_T,
            "w8": np.ascontiguousarray(w_q[:, sl]),
            "sc": sc,
            "cb": cb,
        })
    return in_maps


def _run(in_maps, **kwargs):
    nc = _build()
    return bass_utils.run_bass_kernel_spmd(
        nc, in_maps, core_ids=list(range(NCORES)), **kwargs)


def kernel(x, weight, split_square_len):
    in_maps = _prepare_inputs(x, weight, split_square_len)
    res = _run(in_maps)
    outT = np.concatenate([res.results[c]["outT"] for c in range(NCORES)],
                          axis=0)          # [N, B]
    return outT.T                          # [B, N] view
